# revision 1
# baseline (speedup 1.0000x reference)
"""GAT (2-layer, multi-head) Bass kernel for Trainium2, 8 NeuronCores.

Sharding: core c handles batch b = c // 4 and query-row block iblk = c % 4
(512 rows). Layer 1 computes all 4 heads for those rows; layer 2 follows a
4-core AllGather of the per-core h2 features.

Math trick: with u = s1_i + s2_j,  exp(leakyrelu(u)) = max(exp(u), exp(0.2 u))
         = max(A_i*B_j, a_i*b_j)   (A=exp(s1), B=exp(s2), a=exp(.2 s1), ...)
so the N^2 score matrix needs no transcendentals on the big tiles — just
tensor_scalar / scalar_tensor_tensor / tensor_tensor ops, plus optional
ACT-engine exp route for load balancing. Softmax denominators come from an
extra ones-column in the value matrix; normalization is deferred to the
[rows, 65] matmul output. No stability shift is needed: |s| stays far below
exp overflow (verified in test harness).
"""
import sys

import numpy as np

sys.path.insert(0, "/opt/trn_rl_repo")

import ml_dtypes  # noqa: E402
import concourse.bass as bass  # noqa: E402
import concourse.mybir as mybir  # noqa: E402
import concourse.tile as tile  # noqa: E402
from concourse import bacc, bass_utils  # noqa: E402
from concourse.masks import make_identity  # noqa: E402

BF = ml_dtypes.bfloat16
AF = mybir.ActivationFunctionType
OP = mybir.AluOpType
DT = mybir.dt
AX = mybir.AxisListType

CFG = dict(N=2048, B=2, H=4, F=128, HID=64, OUT=64, CORES=8)
ALPHA = 0.2
# every ACT_EVERY-th (head, jtile) macro-tile computes the two exps on the
# scalar engine instead of the DVE factorized path (engine load balancing)
ACT_EVERY = 2


def _bcast_ap(row_ap, parts=128):
    """DRAM row [1, F] -> broadcast AP [[0, parts], [1, F]] for DMA."""
    return bass.AP(
        tensor=row_ap.tensor,
        offset=row_ap.offset,
        ap=[[0, parts]] + [list(d) for d in row_ap.ap[1:]],
    )


def build_nc(C=CFG):
    N, B, H, F, HID, OUTD, CORES = (
        C["N"], C["B"], C["H"], C["F"], C["HID"], C["OUT"], C["CORES"])
    R = N * B // CORES        # query rows per core
    JT = N // 128             # j (key) tiles
    IS = R // 128             # islices of the row block
    G = CORES // B            # cores per batch
    HW = HID + H              # head slot width in hp2 (ones col at HID + h)
    no_tp = C.get("no_tp", False)   # bisect: skip PE transposes
    no_bc = C.get("no_bc", False)   # bisect: skip broadcast DMAs
    no_gp = C.get("no_gp", False)   # bisect: route gpsimd TT/memset to DVE
    f32, bf16 = DT.float32, DT.bfloat16

    nc = bacc.Bacc("TRN2", num_devices=CORES, debug=False)

    xT = nc.dram_tensor("xT", [F, N], f32, kind="ExternalInput")
    xTmy = nc.dram_tensor("xTmy", [F, R], f32, kind="ExternalInput")
    xTb = nc.dram_tensor("xTb", [F, N], bf16, kind="ExternalInput")
    muT = nc.dram_tensor("muT", [N, R], bf16, kind="ExternalInput")
    w_all = nc.dram_tensor("w_all", [F, H * HID], bf16, kind="ExternalInput")
    wa = nc.dram_tensor("wa", [F, 2 * H], f32, kind="ExternalInput")
    wf = nc.dram_tensor("wf", [H * HID, OUTD], bf16, kind="ExternalInput")
    afr = nc.dram_tensor("afr", [128, 2 * OUTD], bf16, kind="ExternalInput")
    a1fc = nc.dram_tensor("a1fc", [OUTD, 1], bf16, kind="ExternalInput")
    out_d = nc.dram_tensor("out", [R, OUTD], f32, kind="ExternalOutput")

    dbg = C.get("dbg", False)
    if dbg:
        d_sall = nc.dram_tensor("d_sall", [128, JT * 2 * H], f32, kind="ExternalOutput")
        d_ball = nc.dram_tensor("d_ball", [128, 2 * JT * 2 * H], f32, kind="ExternalOutput")
        d_rows1 = nc.dram_tensor("d_rows1", [2 * H, R], bf16, kind="ExternalOutput")
        d_den = nc.dram_tensor("d_den", [H, R], f32, kind="ExternalOutput")
        d_r4 = nc.dram_tensor("d_r4", [H, R], f32, kind="ExternalOutput")
        d_cat = nc.dram_tensor("d_cat", [2 * 128, R], bf16, kind="ExternalOutput")
        d_h2p = nc.dram_tensor("d_h2p", [128, JT * (OUTD + 1)], bf16, kind="ExternalOutput")
        d_s2f = nc.dram_tensor("d_s2f", [128, JT], f32, kind="ExternalOutput")
        d_oT2 = nc.dram_tensor("d_oT2", [OUTD + 1, R], f32, kind="ExternalOutput")
        d_arep = nc.dram_tensor("d_arep", [128, R], bf16, kind="ExternalOutput")
    rows1_d = nc.dram_tensor("rows1_d", [6 * H, R], bf16)
    rows2_d = nc.dram_tensor("rows2_d", [3, R], bf16)
    rdrm = nc.dram_tensor("rdrm", [H, R], f32)
    h2_in = nc.dram_tensor("h2_in", [R, OUTD], bf16)
    h2_out = nc.dram_tensor("h2_out", [G * R, OUTD], bf16)

    with tile.TileContext(nc) as tc:
        with (
            tc.tile_pool(name="const", bufs=1) as pc,
            tc.tile_pool(name="mu", bufs=1) as pmu,
            tc.tile_pool(name="hp", bufs=1) as php,
            tc.tile_pool(name="reps", bufs=2) as prep,
            tc.tile_pool(name="misc", bufs=1) as pm,
            tc.tile_pool(name="work", bufs=16) as pw,
            tc.tile_pool(name="small", bufs=8) as ps,
            tc.tile_pool(name="pacc", bufs=1, space="PSUM") as pacc,
            tc.tile_pool(name="pmm", bufs=2, space="PSUM") as pmm,
            tc.tile_pool(name="pmm2", bufs=2, space="PSUM") as pmm2,
        ):
            # ---- constants / inputs resident in SBUF ----
            ident = pc.tile([128, 128], f32, tag="ident")
            make_identity(nc, ident[:, :])
            xT_sb = pc.tile([F, N], f32, tag="xT")
            nc.sync.dma_start(out=xT_sb[:, :], in_=xT[:, :])
            xTmy_sb = pc.tile([F, R], f32, tag="xTmy")
            nc.sync.dma_start(out=xTmy_sb[:, :], in_=xTmy[:, :])
            xTb_sb = pc.tile([F, N], bf16, tag="xTb")
            nc.sync.dma_start(out=xTb_sb[:, :], in_=xTb[:, :])
            w_sb = pc.tile([F, H * HID], bf16, tag="w")
            nc.sync.dma_start(out=w_sb[:, :], in_=w_all[:, :])
            wa_sb = pc.tile([F, 2 * H], f32, tag="wa")
            nc.sync.dma_start(out=wa_sb[:, :], in_=wa[:, :])
            wf_sb = [pc.tile([128, OUTD], bf16, tag=f"wf{i}", name=f"wf{i}") for i in range(2)]
            for i in range(2):
                nc.sync.dma_start(out=wf_sb[i][:, :], in_=wf[128 * i:128 * (i + 1), :])
            afr_sb = pc.tile([128, 2 * OUTD], bf16, tag="afr")
            nc.sync.dma_start(out=afr_sb[:, :], in_=afr[:, :])
            a1f_sb = pc.tile([OUTD, 1], bf16, tag="a1f")
            nc.sync.dma_start(out=a1f_sb[:, :], in_=a1fc[:, :])

            mu_sb = []
            for t in range(JT):
                m = pmu.tile([128, R], bf16, tag=f"mu{t}", name=f"mu{t}")
                nc.sync.dma_start(out=m[:, :], in_=muT[128 * t:128 * (t + 1), :])
                mu_sb.append(m)

            # ---- phase 1: s vectors ----
            # per-partition (column) form for all j: s_all[:, 2h]=s1, [:,2h+1]=s2
            s_all = pm.tile([128, JT * 2 * H], f32, tag="s_all")
            for t in range(JT):
                ps_s = pmm2.tile([128, 2 * H], f32, tag="sm", name="ps_s")
                nc.tensor.matmul(ps_s[:, :], lhsT=xT_sb[:, 128 * t:128 * (t + 1)],
                                 rhs=wa_sb[:, :], start=True, stop=True)
                nc.vector.tensor_copy(s_all[:, 2 * H * t:2 * H * (t + 1)], ps_s[:, :])
            s02_all = pm.tile([128, JT * 2 * H], f32, tag="s02_all")
            nc.vector.tensor_scalar_mul(s02_all[:, :], s_all[:, :], ALPHA)
            B_all = pm.tile([128, 2 * JT * 2 * H], f32, tag="B_all")
            nhalf = JT * 2 * H
            nc.scalar.activation(B_all[:, 0:nhalf], s_all[:, :], AF.Exp, scale=1.0)
            nc.scalar.activation(B_all[:, nhalf:2 * nhalf], s_all[:, :], AF.Exp,
                                 scale=ALPHA)

            # row form for this core's query rows: psum_sr[2h] = s1_h rows
            ps_sr = pmm.tile([2 * H, R], f32, tag="big", name="ps_sr")
            nc.tensor.matmul(ps_sr[:, :], lhsT=wa_sb[:, :], rhs=xTmy_sb[:, :],
                             start=True, stop=True)
            rows1A = pm.tile([2 * H, R], bf16, tag="rows1A")
            rows1a = pm.tile([2 * H, R], bf16, tag="rows1a")
            rows1s = pm.tile([2 * H, R], bf16, tag="rows1s")
            nc.scalar.activation(rows1A[:, :], ps_sr[:, :], AF.Exp, scale=1.0)
            nc.scalar.activation(rows1a[:, :], ps_sr[:, :], AF.Exp, scale=ALPHA)
            nc.scalar.copy(rows1s[:, :], ps_sr[:, :])
            nc.gpsimd.dma_start(out=rows1_d[0:2 * H, :], in_=rows1A[:, :])
            nc.gpsimd.dma_start(out=rows1_d[2 * H:4 * H, :], in_=rows1a[:, :])
            nc.gpsimd.dma_start(out=rows1_d[4 * H:6 * H, :], in_=rows1s[:, :])
            A_rep, a_rep, s1_rep = [], [], []
            for h in range(H):
                Ar = prep.tile([128, R], bf16, tag=f"Ar{h}", name=f"Ar{h}")
                ar = prep.tile([128, R], bf16, tag=f"ar{h}", name=f"ar{h}")
                sr = prep.tile([128, R], bf16, tag=f"sr{h}", name=f"sr{h}")
                if not no_bc:
                    nc.gpsimd.dma_start(out=Ar[:, :], in_=_bcast_ap(rows1_d.ap()[2 * h:2 * h + 1, :]))
                    nc.gpsimd.dma_start(out=ar[:, :], in_=_bcast_ap(rows1_d.ap()[2 * H + 2 * h:2 * H + 2 * h + 1, :]))
                    nc.gpsimd.dma_start(out=sr[:, :], in_=_bcast_ap(rows1_d.ap()[4 * H + 2 * h:4 * H + 2 * h + 1, :]))
                else:
                    nc.vector.memset(Ar[:, :], 1.0)
                    nc.vector.memset(ar[:, :], 1.0)
                    nc.vector.memset(sr[:, :], 0.0)
                A_rep.append(Ar)
                a_rep.append(ar)
                s1_rep.append(sr)

            if dbg:
                nc.sync.dma_start(out=d_sall[:, :], in_=s_all[:, :])
                nc.sync.dma_start(out=d_ball[:, :], in_=B_all[:, :])
                nc.sync.dma_start(out=d_rows1[:, :], in_=rows1A[:, :])
                nc.sync.dma_start(out=d_arep[:, :], in_=A_rep[0][:, :])

            # ---- phase 2: hp2 = [h_head | ones] per head, bf16 ----
            hp2 = []
            for t in range(JT):
                hp = php.tile([128, H * HW], bf16, tag=f"hp{t}", name=f"hp{t}")
                hp2.append(hp)
            for t in range(JT):
                ps_h = pmm.tile([128, H * HID], f32, tag="big", name="ps_h")
                nc.tensor.matmul(ps_h[:, :], lhsT=xTb_sb[:, 128 * t:128 * (t + 1)],
                                 rhs=w_sb[:, :], start=True, stop=True)
                hview = hp2[t].rearrange("p (g c) -> p g c", c=HW)
                nc.scalar.copy(hview[:, :, 0:HID],
                               ps_h[:, :].rearrange("p (g c) -> p g c", c=HID))
                (nc.vector if no_gp else nc.gpsimd).memset(hview[:, :, HID:HW], 0.0)
                for h in range(H):
                    (nc.vector if no_gp else nc.gpsimd).memset(hp2[t][:, HW * h + HID + h:HW * h + HID + h + 1], 1.0)

            # ---- phase 3: layer-1 attention, head-major ----
            acc = []
            for h in range(H):
                ac = pacc.tile([HID + H, R], f32, tag=f"acc{h}", name=f"acc{h}")
                acc.append(ac)
            for t in range(JT):
                for h in range(H):
                    idx = h * JT + t
                    wm = pw.tile([128, R], bf16, tag="wm")
                    Bap = B_all[:, 2 * H * t + 2 * h + 1:2 * H * t + 2 * h + 2]
                    bap = B_all[:, nhalf + 2 * H * t + 2 * h + 1:nhalf + 2 * H * t + 2 * h + 2]
                    if idx % 3 != 0:  # ACT route (2/3 of tiles)
                        s2ap = s_all[:, 2 * H * t + 2 * h + 1:2 * H * t + 2 * h + 2]
                        s02ap = s02_all[:, 2 * H * t + 2 * h + 1:2 * H * t + 2 * h + 2]
                        t1 = pw.tile([128, R], bf16, tag="t1")
                        t2 = pw.tile([128, R], bf16, tag="t2")
                        nc.scalar.activation(t1[:, :], s1_rep[h][:, :], AF.Exp,
                                             bias=s2ap, scale=1.0)
                        nc.scalar.activation(t2[:, :], s1_rep[h][:, :], AF.Exp,
                                             bias=s02ap, scale=ALPHA)
                        w_t = pw.tile([128, R], bf16, tag="wt")
                        nc.vector.tensor_max(w_t[:, :], t1[:, :], t2[:, :])
                        nc.vector.tensor_mul(wm[:, :], w_t[:, :], mu_sb[t][:, :])
                    else:  # DVE factorized route
                        t2 = pw.tile([128, R], bf16, tag="t2")
                        nc.vector.tensor_scalar_mul(t2[:, :], a_rep[h][:, :], bap)
                        w_t = pw.tile([128, R], bf16, tag="wt")
                        nc.vector.scalar_tensor_tensor(
                            w_t[:, :], A_rep[h][:, :], Bap, t2[:, :],
                            op0=OP.mult, op1=OP.max)
                        nc.vector.tensor_mul(wm[:, :], w_t[:, :], mu_sb[t][:, :])
                    nc.tensor.matmul(acc[h][:, :],
                                     lhsT=hp2[t][:, HW * h:HW * (h + 1)],
                                     rhs=wm[:, :], start=(t == 0), stop=(t == JT - 1))

            # ---- normalization: denominators live at partition HID+h ----
            den4 = pm.tile([HID + H, R], f32, tag="den4")
            nc.scalar.copy(den4[HID:HID + H, :], acc[0][HID:HID + H, :])
            for h in range(1, H):
                nc.vector.tensor_add(den4[HID:HID + H, :], den4[HID:HID + H, :],
                                     acc[h][HID:HID + H, :])
            r4 = pm.tile([H, R], f32, tag="r4")
            if not no_tp:
                for i in range(IS):
                    ptp = pmm2.tile([128, H], f32, tag="sm", name="ptp")
                    nc.tensor.transpose(ptp[:, :], den4[HID:HID + H, 128 * i:128 * (i + 1)],
                                        ident[HID:HID + H, HID:HID + H])
                    rsb = ps.tile([128, H], f32, tag="rsb")
                    nc.vector.reciprocal(rsb[:, :], ptp[:, :])
                    ptr = pmm2.tile([H, 128], f32, tag="sm", name="ptr")
                    nc.tensor.transpose(ptr[:, :], rsb[:, :], ident[:, :])
                    nc.scalar.copy(r4[:, 128 * i:128 * (i + 1)], ptr[:, :])
            else:
                nc.vector.memset(r4[:, :], 1.0)
            nc.gpsimd.dma_start(out=rdrm[:, :], in_=r4[:, :])
            if dbg:
                nc.sync.dma_start(out=d_den[:, :], in_=den4[HID:HID + H, :])
                nc.sync.dma_start(out=d_r4[:, :], in_=r4[:, :])
            catT = [pm.tile([128, R], bf16, tag=f"catT{i}", name=f"catT{i}") for i in range(2)]
            for h in range(H):
                rr = prep.tile([128, R], f32, tag="rrep")
                if not no_bc:
                    nc.gpsimd.dma_start(out=rr[:, :], in_=_bcast_ap(rdrm.ap()[h:h + 1, :]))
                else:
                    nc.vector.memset(rr[:, :], 1.0)
                nc.vector.tensor_mul(catT[h // 2][64 * (h % 2):64 * (h % 2) + 64, :],
                                     acc[h][0:HID, :], rr[0:HID, :])
            # ELU on catT (bf16, elementwise, layout-free)
            for i in range(2):
                mclamp = pw.tile([128, R], bf16, tag="mclamp")
                nc.vector.tensor_scalar_min(mclamp[:, :], catT[i][:, :], 0.0)
                ee = pw.tile([128, R], bf16, tag="ee")
                nc.scalar.activation(ee[:, :], mclamp[:, :], AF.Exp)
                nc.vector.tensor_scalar_sub(ee[:, :], ee[:, :], 1.0)
                nc.vector.tensor_max(catT[i][:, :], catT[i][:, :], ee[:, :])

            if dbg:
                for _i in range(2):
                    nc.sync.dma_start(out=d_cat[128 * _i:128 * (_i + 1), :],
                                      in_=catT[_i][:, :])

            # ---- phase 4: h2 local, h2T, s1f, allgather ----
            h2loc = [pm.tile([128, OUTD], bf16, tag=f"h2l{i}", name=f"h2l{i}") for i in range(IS)]
            for i in range(IS):
                ph2 = pmm2.tile([128, OUTD], f32, tag="sm", name="ph2")
                for ct in range(2):
                    nc.tensor.matmul(ph2[:, :],
                                     lhsT=catT[ct][:, 128 * i:128 * (i + 1)],
                                     rhs=wf_sb[ct][:, :],
                                     start=(ct == 0), stop=(ct == 1))
                nc.vector.tensor_copy(h2loc[i][:, :], ph2[:, :])
                nc.sync.dma_start(out=h2_in[128 * i:128 * (i + 1), :], in_=h2loc[i][:, :])
            ph2T = pmm.tile([OUTD, R], f32, tag="big", name="ph2T")
            for ct in range(2):
                nc.tensor.matmul(ph2T[:, :], lhsT=wf_sb[ct][:, :],
                                 rhs=catT[ct][:, :], start=(ct == 0), stop=(ct == 1))
            h2T_sb = pm.tile([OUTD, R], bf16, tag="h2T")
            nc.vector.tensor_copy(h2T_sb[:, :], ph2T[:, :])
            ps1f = pmm2.tile([1, R], f32, tag="sm", name="ps1f")
            nc.tensor.matmul(ps1f[:, :], lhsT=a1f_sb[:, :], rhs=h2T_sb[:, :],
                             start=True, stop=True)
            rows2A = pm.tile([1, R], bf16, tag="rows2A")
            rows2a = pm.tile([1, R], bf16, tag="rows2a")
            rows2s = pm.tile([1, R], bf16, tag="rows2s")
            nc.scalar.activation(rows2A[:, :], ps1f[:, :], AF.Exp, scale=1.0)
            nc.scalar.activation(rows2a[:, :], ps1f[:, :], AF.Exp, scale=ALPHA)
            nc.scalar.copy(rows2s[:, :], ps1f[:, :])
            nc.gpsimd.dma_start(out=rows2_d[0:1, :], in_=rows2A[:, :])
            nc.gpsimd.dma_start(out=rows2_d[1:2, :], in_=rows2a[:, :])
            nc.gpsimd.dma_start(out=rows2_d[2:3, :], in_=rows2s[:, :])
            A2_rep = prep.tile([128, R], bf16, tag="A2")
            a2_rep = prep.tile([128, R], bf16, tag="a2")
            s1f_rep = prep.tile([128, R], bf16, tag="s1f")
            if not no_bc:
                nc.gpsimd.dma_start(out=A2_rep[:, :], in_=_bcast_ap(rows2_d.ap()[0:1, :]))
                nc.gpsimd.dma_start(out=a2_rep[:, :], in_=_bcast_ap(rows2_d.ap()[1:2, :]))
                nc.gpsimd.dma_start(out=s1f_rep[:, :], in_=_bcast_ap(rows2_d.ap()[2:3, :]))
            else:
                nc.vector.memset(A2_rep[:, :], 1.0)
                nc.vector.memset(a2_rep[:, :], 1.0)
                nc.vector.memset(s1f_rep[:, :], 0.0)

            groups = [list(range(G * b, G * (b + 1))) for b in range(CORES // G)]
            nc.gpsimd.collective_compute(
                "AllGather", OP.bypass, replica_groups=groups,
                ins=[h2_in.ap().opt()], outs=[h2_out.ap().opt()])

            # gathered h2 -> [128, JT*65] with ones cols
            O1 = OUTD + 1
            h2p = pm.tile([128, JT * O1], bf16, tag="h2p")
            h2pv = h2p.rearrange("p (t c) -> p t c", c=O1)
            nc.sync.dma_start(
                out=h2pv[:, :, 0:OUTD],
                in_=h2_out.ap().rearrange("(t p) k -> p t k", p=128))
            (nc.vector if no_gp else nc.gpsimd).memset(h2pv[:, :, OUTD:O1], 1.0)

            # s2f per-partition + exps
            s2f = pm.tile([128, JT], f32, tag="s2f")
            s02f = pm.tile([128, JT], f32, tag="s02f")
            for t in range(JT):
                scr = ps.tile([128, OUTD], bf16, tag="scr")
                nc.vector.tensor_mul(scr[:, :], h2pv[:, t, 0:OUTD],
                                     afr_sb[:, OUTD:2 * OUTD])
                nc.vector.tensor_reduce(s2f[:, t:t + 1], scr[:, :],
                                        axis=AX.X, op=OP.add)
            nc.vector.tensor_scalar_mul(s02f[:, :], s2f[:, :], ALPHA)
            B2 = pm.tile([128, 2 * JT], f32, tag="B2")
            nc.scalar.activation(B2[:, 0:JT], s2f[:, :], AF.Exp, scale=1.0)
            nc.scalar.activation(B2[:, JT:2 * JT], s2f[:, :], AF.Exp, scale=ALPHA)

            if dbg:
                nc.sync.dma_start(out=d_h2p[:, :], in_=h2p[:, :])
                nc.sync.dma_start(out=d_s2f[:, :], in_=s2f[:, :])

            # ---- phase 5: layer-2 attention ----
            acc2 = pacc.tile([OUTD + 1, R], f32, tag="acc0", name="accB")
            for t in range(JT):
                wm = pw.tile([128, R], bf16, tag="wm")
                if t % 3 != 0:
                    t1 = pw.tile([128, R], bf16, tag="t1")
                    t2 = pw.tile([128, R], bf16, tag="t2")
                    nc.scalar.activation(t1[:, :], s1f_rep[:, :], AF.Exp,
                                         bias=s2f[:, t:t + 1], scale=1.0)
                    nc.scalar.activation(t2[:, :], s1f_rep[:, :], AF.Exp,
                                         bias=s02f[:, t:t + 1], scale=ALPHA)
                    w_t = pw.tile([128, R], bf16, tag="wt")
                    nc.vector.tensor_max(w_t[:, :], t1[:, :], t2[:, :])
                    nc.vector.tensor_mul(wm[:, :], w_t[:, :], mu_sb[t][:, :])
                else:
                    t2 = pw.tile([128, R], bf16, tag="t2")
                    nc.vector.tensor_scalar_mul(t2[:, :], a2_rep[:, :],
                                                B2[:, JT + t:JT + t + 1])
                    w_t = pw.tile([128, R], bf16, tag="wt")
                    nc.vector.scalar_tensor_tensor(
                        w_t[:, :], A2_rep[:, :], B2[:, t:t + 1], t2[:, :],
                        op0=OP.mult, op1=OP.max)
                    nc.vector.tensor_mul(wm[:, :], w_t[:, :], mu_sb[t][:, :])
                nc.tensor.matmul(acc2[:, :], lhsT=h2p[:, O1 * t:O1 * (t + 1)],
                                 rhs=wm[:, :], start=(t == 0), stop=(t == JT - 1))

            # ---- final: transpose to [i, 65], normalize, ELU, log_softmax ----
            oT2 = pm.tile([O1, R], f32, tag="oT2")
            nc.vector.tensor_copy(oT2[:, :], acc2[:, :])
            if dbg:
                nc.sync.dma_start(out=d_oT2[:, :], in_=oT2[:, :])
            for i in range(IS):
                pt = pmm2.tile([128, O1], f32, tag="sm", name="pt")
                if not no_tp:
                    nc.tensor.transpose(pt[:, :], oT2[:, 128 * i:128 * (i + 1)],
                                        ident[0:O1, 0:O1])
                else:
                    nc.vector.memset(pt[:, :], 1.0)
                r2 = ps.tile([128, 1], f32, tag="r2")
                nc.vector.reciprocal(r2[:, :], pt[:, OUTD:O1])
                of = ps.tile([128, OUTD], f32, tag="of")
                nc.vector.tensor_scalar_mul(of[:, :], pt[:, 0:OUTD], r2[:, 0:1])
                mc = ps.tile([128, OUTD], f32, tag="mc")
                nc.vector.tensor_scalar_min(mc[:, :], of[:, :], 0.0)
                ee = ps.tile([128, OUTD], f32, tag="ee2")
                nc.scalar.activation(ee[:, :], mc[:, :], AF.Exp)
                nc.vector.tensor_scalar_sub(ee[:, :], ee[:, :], 1.0)
                oe = ps.tile([128, OUTD], f32, tag="oe")
                nc.vector.tensor_max(oe[:, :], of[:, :], ee[:, :])
                mx = ps.tile([128, 1], f32, tag="mx")
                nc.vector.tensor_reduce(mx[:, :], oe[:, :], axis=AX.X, op=OP.max)
                xm = ps.tile([128, OUTD], f32, tag="xm")
                nc.vector.tensor_scalar_sub(xm[:, :], oe[:, :], mx[:, 0:1])
                ex = ps.tile([128, OUTD], f32, tag="ex")
                sm = ps.tile([128, 1], f32, tag="sm")
                nc.scalar.activation(ex[:, :], xm[:, :], AF.Exp, accum_out=sm[:, :])
                ln = ps.tile([128, 1], f32, tag="ln")
                nc.scalar.activation(ln[:, :], sm[:, :], AF.Ln)
                res = ps.tile([128, OUTD], f32, tag="res")
                nc.vector.tensor_scalar_sub(res[:, :], xm[:, :], ln[:, 0:1])
                nc.sync.dma_start(out=out_d[128 * i:128 * (i + 1), :], in_=res[:, :])

    nc.compile()
    return nc


def host_prep(nodes, adj, W, a1, a2, Wf, a1f, a2f, C=CFG):
    """Build per-core input maps. Layout/dtype prep only (plus folding the
    attention vectors into W, a weights-only transform)."""
    N, B, H, F, HID, OUTD, CORES = (
        C["N"], C["B"], C["H"], C["F"], C["HID"], C["OUT"], C["CORES"])
    R = N * B // CORES
    G = CORES // B
    nodes = np.asarray(nodes, np.float32)
    adj = np.asarray(adj)
    W = np.asarray(W, np.float32)
    a1 = np.asarray(a1, np.float32)
    a2 = np.asarray(a2, np.float32)
    Wf = np.asarray(Wf, np.float32)
    a1f = np.asarray(a1f, np.float32)
    a2f = np.asarray(a2f, np.float32)

    w_all = np.concatenate([W[h] for h in range(H)], axis=1).astype(BF)  # [F, H*HID]
    wa_cols = []
    for h in range(H):
        wa_cols.append(W[h] @ a1[h])
        wa_cols.append(W[h] @ a2[h])
    wa = np.stack(wa_cols, axis=1).astype(np.float32)  # [F, 2H]
    wf_b = Wf.astype(BF)
    afr = np.zeros((128, 2 * OUTD), np.float32)
    afr[:, 0:OUTD] = a1f[None, :]
    afr[:, OUTD:2 * OUTD] = a2f[None, :]
    afr = afr.astype(BF)
    a1fc = a1f[:, None].astype(BF)

    in_maps = []
    for c in range(CORES):
        b, iblk = c // G, c % G
        xTb_full = np.ascontiguousarray(nodes[b].T)  # [F, N]
        muT = np.ascontiguousarray(
            (adj[b] != 0).T[:, R * iblk:R * (iblk + 1)]).astype(BF)
        in_maps.append({
            "xT": xTb_full.astype(np.float32),
            "xTmy": np.ascontiguousarray(xTb_full[:, R * iblk:R * (iblk + 1)]).astype(np.float32),
            "xTb": xTb_full.astype(BF),
            "muT": muT,
            "w_all": w_all,
            "wa": wa,
            "wf": wf_b,
            "afr": afr,
            "a1fc": a1fc,
        })
    return in_maps


_NC_CACHE = {}


def kernel(nodes, adj, W, a1, a2, Wf, a1f, a2f):
    C = CFG
    key = "main"
    if key not in _NC_CACHE:
        _NC_CACHE[key] = build_nc(C)
    nc = _NC_CACHE[key]
    in_maps = host_prep(nodes, adj, W, a1, a2, Wf, a1f, a2f, C)
    res = bass_utils.run_bass_kernel_spmd(
        nc, in_maps, core_ids=list(range(C["CORES"])))
    N, B, CORES, OUTD = C["N"], C["B"], C["CORES"], C["OUT"]
    R = N * B // CORES
    G = CORES // B
    out = np.zeros((B, N, OUTD), np.float32)
    for c in range(CORES):
        b, iblk = c // G, c % G
        out[b, R * iblk:R * (iblk + 1), :] = res.results[c]["out"]
    return out

